# revision 1
# baseline (speedup 1.0000x reference)
"""Trainium2 Bass kernel for nn_Bert_Traj_Model (12-layer BERT-style encoder).

Sharding: pure data-parallel over batch. 32 sequences -> 4 per core x 8 cores.
Each core runs the full 12-layer transformer on its 4 sequences (2048 tokens).
No collectives; the host splits inputs and concatenates outputs.

Device layout: residual stream kept TRANSPOSED, hT[d_model(6x128 part tiles),
token], so every matmul's contraction dim (d) sits on partitions. Per-token
scalars (LN mean/rstd, softmax denominators) are produced as [1, T] rows via
ones-vector matmuls (partition reduction on the PE) and broadcast back across
partitions via K=1 outer-product matmuls. Dense matmuls run in f32r.

LayerNorm folding: gains are folded into the following weights on the host;
mean subtraction folds into each matmul as a rank-1 correction
((h - 1 x m) @ W = h @ W + (-m) x colsum(W)); rstd is applied to matmul
outputs (it commutes through the contraction per-token).

Attention: scores computed transposed S^T[k, q] per (seq, head); causal/
prefix mask added in PSUM via identity-matmul on diagonal 128x128 tiles only
(other tiles are fully live or fully masked -> skipped); padding mask is a
per-partition bias on the Exp activation. Softmax runs without max
subtraction (pre-norm LN bounds the scores). The denominator comes free from
an appended ones-column on V ([V|1] stationary); normalization is a
reciprocal row + outer-product broadcast + one multiply.
"""

import numpy as np
from contextlib import ExitStack

import ml_dtypes

import concourse.bass as bass  # noqa: F401  (kept for users of this module)
import concourse.bacc as bacc
import concourse.tile as tile
import concourse.mybir as mybir
from concourse import bass_utils

# ---------------- model constants (hardcoded per spec) ----------------
D = 768
H = 12
DH = 64
FF = 3072
S = 512
B_GLOBAL = 32
N_CORES = 8
BL = B_GLOBAL // N_CORES      # 4 sequences per core
T = BL * S                    # 2048 tokens per core
KT = D // 128                 # 6 d_model partition tiles
FT = FF // 128                # 24 d_ff partition tiles
G16 = T // 128                # 16 token slices per core
NL = 12
EPS = 1e-6
NEGM = -1.0e5                 # additive mask value
TOKV = 10000
TIMV = 48

F32 = mybir.dt.float32
F32R = mybir.dt.float32r
BF16 = mybir.dt.bfloat16
I16 = mybir.dt.int16
AF = mybir.ActivationFunctionType
OP = mybir.AluOpType


def _r(ap):
    return ap.bitcast(F32R)


def build_nc(n_layers=NL, has_bo=False, has_b2=False, has_embgb=False,
             dbg_h=False):
    """Build and compile the per-core Bass program."""
    nc = bacc.Bacc("TRN2", target_bir_lowering=False, debug=False)

    d = {}
    def din(name, shape, dt):
        d[name] = nc.dram_tensor(name, shape, dt, kind="ExternalInput").ap()

    din("tok_emb", [TOKV, D], F32)
    din("tim_emb", [TIMV, D], F32)
    din("tok_idx", [128, T // 16], I16)
    din("tim_idx", [128, T // 16], I16)
    din("posT", [128, KT, S], F32)
    din("padadd", [128, G16], F32)
    din("mask00", [128, 128], BF16)
    din("maskdg", [128, 128], BF16)
    din("i128b", [128, 128], BF16)
    din("i128f", [128, 128], F32)
    din("wq", [n_layers, KT, 128, KT, 128], F32R)
    din("wk", [n_layers, KT, 128, KT, 128], F32R)
    din("wv", [n_layers, 128, KT, D], F32R)
    din("wo", [n_layers, 128, KT, D], F32R)
    din("w1t", [n_layers, FT, 128, KT, 128], F32R)
    din("w2t", [n_layers, FT, 128, D], F32R)
    din("csqkv", [n_layers, 1, 3 * D], F32R)
    din("cs1", [n_layers, 1, FF], F32R)
    if has_bo:
        din("bo_c", [128, n_layers, KT], F32)
    if has_b2:
        din("b2_c", [128, n_layers, KT], F32)
    if has_embgb:
        din("embg", [128, KT, 2], F32)
    d_out = nc.dram_tensor("out", [T, D], F32, kind="ExternalOutput").ap()
    if dbg_h:
        d["dbg_h"] = nc.dram_tensor("dbg_h", [128, KT, T], F32,
                                    kind="ExternalOutput").ap()

    with tile.TileContext(nc) as tc:
        _emit(tc, n_layers, has_bo, has_b2, has_embgb, d, d_out)

    nc.compile()
    return nc


def _ln_stats(nc, pspool, sqpool, rowpool, ones_col, h, cs, nm_tag, rs_tag,
              eps_row, scrpool, pstag="stats"):
    """LN stats over the partition axis for h[:, :, cs].
    Returns SBUF rows nm = -mean [1, S], rstd [1, S].
    (f32r matmuls must write psum base partition 0 -> two separate tiles.)"""
    st_s = pspool.tile([2, S], F32, tag=pstag)
    st_q = pspool.tile([2, S], F32, tag=pstag)
    for kt in range(KT):
        sq = sqpool.tile([128, S], F32R, tag="sq")
        nc.scalar.activation(sq[:], h[:, kt, cs], AF.Square)
        nc.tensor.matmul(st_s[:], _r(ones_col[:]), _r(h[:, kt, cs]),
                         start=(kt == 0), stop=(kt == KT - 1))
        nc.tensor.matmul(st_q[:], _r(ones_col[:]), _r(sq[:]),
                         start=(kt == 0), stop=(kt == KT - 1))
    nm = rowpool.tile([1, S], F32R, tag=nm_tag)
    nc.vector.tensor_scalar(nm[:], st_s[0:1, :], -1.0 / D, None, OP.mult)
    vs = scrpool.tile([1, S], F32, tag="scr")
    nc.vector.tensor_scalar(vs[:], st_q[0:1, :], 1.0 / D, None, OP.mult)
    m2 = scrpool.tile([1, S], F32, tag="scr")
    nc.vector.tensor_tensor(m2[:], nm[:], nm[:], OP.mult)
    var = scrpool.tile([1, S], F32, tag="scr")
    nc.vector.tensor_tensor(var[:], vs[:], m2[:], OP.subtract)
    lnv = scrpool.tile([1, S], F32, tag="scr")
    nc.scalar.activation(lnv[:], var[:], AF.Ln, bias=eps_row[0:1, 0:1])
    rstd = rowpool.tile([1, S], F32R, tag=rs_tag)
    nc.scalar.activation(rstd[:], lnv[:], AF.Exp, scale=-0.5)
    return nm, rstd


def _emit(tc, n_layers, has_bo, has_b2, has_embgb, d, d_out):
    nc = tc.nc
    with ExitStack() as ctx:
        # ---------------- persistent pools ----------------
        cpool = ctx.enter_context(tc.tile_pool(name="const", bufs=1))
        rowp2 = ctx.enter_context(tc.tile_pool(name="rowp2", bufs=2))
        rowp4 = ctx.enter_context(tc.tile_pool(name="rowp4", bufs=4))
        rownm2 = ctx.enter_context(tc.tile_pool(name="rownm2", bufs=BL))
        hpool = ctx.enter_context(tc.tile_pool(name="hres", bufs=1))
        sqpool = ctx.enter_context(tc.tile_pool(name="sq", bufs=2))
        b2spool = ctx.enter_context(tc.tile_pool(name="b2s", bufs=BL))

        onesf = cpool.tile([128, 128], F32, tag="onesf")
        nc.vector.memset(onesf[:], 1.0)
        ones_col = cpool.tile([128, 2], F32R, tag="onec")
        nc.vector.tensor_copy(ones_col[:], onesf[:, 0:2])
        ones_row = cpool.tile([1, 128], F32R, tag="oner")
        nc.vector.tensor_copy(ones_row[:], onesf[0:1, :])
        ident1 = cpool.tile([1, 1], F32, tag="id1")
        nc.vector.memset(ident1[:], 1.0)
        eps_row = cpool.tile([1, 1], F32, tag="eps")
        nc.vector.memset(eps_row[:], EPS)
        i128b = cpool.tile([128, 128], BF16, tag="i128b")
        nc.sync.dma_start(i128b[:], d["i128b"])
        i128f = cpool.tile([128, 128], F32, tag="i128f")
        nc.sync.dma_start(i128f[:], d["i128f"])
        mask00 = cpool.tile([128, 128], BF16, tag="m00")
        nc.sync.dma_start(mask00[:], d["mask00"])
        maskdg = cpool.tile([128, 128], BF16, tag="mdg")
        nc.sync.dma_start(maskdg[:], d["maskdg"])
        padsb = cpool.tile([128, G16], F32, tag="pad")
        nc.sync.dma_start(padsb[:], d["padadd"])

        h = hpool.tile([128, KT, T], F32R, tag="h")
        vaug = cpool.tile([128, BL, H, DH + 1], F32R, tag="vaug")
        nc.vector.tensor_copy(
            vaug[:, :, :, DH:DH + 1],
            onesf[:, 0:BL * H].rearrange("p (b h) -> p b h ()", b=BL))

        bo_c = b2_c = embg = None
        if has_bo:
            bo_c = cpool.tile([128, n_layers, KT], F32, tag="bo")
            nc.sync.dma_start(bo_c[:], d["bo_c"])
        if has_b2:
            b2_c = cpool.tile([128, n_layers, KT], F32, tag="b2")
            nc.sync.dma_start(b2_c[:], d["b2_c"])
        if has_embgb:
            embg = cpool.tile([128, KT, 2], F32, tag="embg")
            nc.sync.dma_start(embg[:], d["embg"])

        # ================= embedding (two halves to bound SBUF) ============
        G8 = G16 // 2
        with tc.tile_pool(name="emb", bufs=1) as ep, \
             tc.tile_pool(name="embps", bufs=6, space="PSUM") as eps2:
            post = ep.tile([128, KT, S], F32, tag="post")
            nc.sync.dma_start(post[:], d["posT"])
            tokidx = ep.tile([128, T // 16], I16, tag="tokidx")
            timidx = ep.tile([128, T // 16], I16, tag="timidx")
            nc.sync.dma_start(tokidx[:], d["tok_idx"])
            nc.sync.dma_start(timidx[:], d["tim_idx"])
            for half in range(2):
                tokn = ep.tile([128, G8, D], F32, tag="tokn")
                timn = ep.tile([128, G8, D], F32, tag="timn")
                isl = slice(half * (T // 32), (half + 1) * (T // 32))
                nc.gpsimd.dma_gather(tokn[:], d["tok_emb"], tokidx[:, isl],
                                     num_idxs=T // 2, num_idxs_reg=T // 2,
                                     elem_size=D)
                nc.gpsimd.dma_gather(timn[:], d["tim_emb"], timidx[:, isl],
                                     num_idxs=T // 2, num_idxs_reg=T // 2,
                                     elem_size=D)
                for gg in range(G8):
                    g = half * G8 + gg
                    sl = (g % BL) * 128
                    for kt in range(KT):
                        p = eps2.tile([128, 128], F32, tag="etp")
                        nc.tensor.matmul(p[:],
                                         tokn[:, gg, kt * 128:(kt + 1) * 128],
                                         i128f[:], is_transpose=True,
                                         start=True, stop=False)
                        nc.tensor.matmul(p[:],
                                         timn[:, gg, kt * 128:(kt + 1) * 128],
                                         i128f[:], is_transpose=True,
                                         start=False, stop=True)
                        nc.vector.tensor_tensor(
                            h[:, kt, g * 128:(g + 1) * 128], p[:],
                            post[:, kt, sl:sl + 128], OP.add)

        # embedding layernorm (writes h in place)
        with tc.tile_pool(name="elnp1", bufs=2, space="PSUM") as lp1, \
             tc.tile_pool(name="elnp2", bufs=2, space="PSUM") as lp2, \
             tc.tile_pool(name="elnsb", bufs=3) as lsb:
            for c in range(BL):
                cs = slice(c * S, (c + 1) * S)
                nm, rstd = _ln_stats(nc, lp1, sqpool, rowp2, ones_col, h, cs,
                                     "nm", "rstd", eps_row, rowp4)
                bcn = lp2.tile([128, S], F32, tag="bc")
                nc.tensor.matmul(bcn[:], _r(ones_row[:]), _r(nm[:]))
                bcr = lp2.tile([128, S], F32, tag="bc")
                nc.tensor.matmul(bcr[:], _r(ones_row[:]), _r(rstd[:]))
                for kt in range(KT):
                    t1 = lsb.tile([128, S], F32, tag="t1")
                    nc.vector.tensor_tensor(t1[:], h[:, kt, cs], bcn[:],
                                            OP.add)
                    if has_embgb:
                        t2 = lsb.tile([128, S], F32, tag="t2")
                        nc.vector.tensor_tensor(t2[:], t1[:], bcr[:], OP.mult)
                        nc.vector.tensor_scalar(
                            h[:, kt, cs], t2[:], embg[:, kt, 0:1],
                            embg[:, kt, 1:2], OP.mult, OP.add)
                    else:
                        nc.vector.tensor_tensor(h[:, kt, cs], t1[:], bcr[:],
                                                OP.mult)

        if "dbg_h" in d:
            nc.sync.dma_start(d["dbg_h"], h[:].bitcast(F32))

        # ================= transformer layers =================
        wvopool = ctx.enter_context(tc.tile_pool(name="wvo", bufs=1))
        cspool = ctx.enter_context(tc.tile_pool(name="cs", bufs=1))

        for lyr in range(n_layers):
            wv = wvopool.tile([128, KT, D], F32R, tag="wv")
            nc.sync.dma_start(wv[:], d["wv"][lyr])
            wo = wvopool.tile([128, KT, D], F32R, tag="wo")
            nc.sync.dma_start(wo[:], d["wo"][lyr])
            cs_sb = cspool.tile([1, 3 * D], F32R, tag="csqkv")
            nc.sync.dma_start(cs_sb[:], d["csqkv"][lyr])

            nm2s, br2s = [], []

            # ---------- phase A: LN1 + QKV + attention + Wo ----------
            with tc.tile_pool(name=f"pB_{lyr}", bufs=2, space="PSUM") as pA2, \
                 tc.tile_pool(name=f"pC_{lyr}", bufs=3, space="PSUM") as pA3, \
                 tc.tile_pool(name=f"wq_{lyr}", bufs=3) as wqp, \
                 tc.tile_pool(name=f"qk_{lyr}", bufs=2) as qkp, \
                 tc.tile_pool(name=f"oT_{lyr}", bufs=1) as oTp, \
                 tc.tile_pool(name=f"at_{lyr}", bufs=4) as atp, \
                 tc.tile_pool(name=f"br_{lyr}", bufs=2) as brp, \
                 tc.tile_pool(name=f"bo_{lyr}", bufs=1) as brp1, \
                 tc.tile_pool(name=f"tm_{lyr}", bufs=2) as tmpA:
                for c in range(BL):
                    cs = slice(c * S, (c + 1) * S)
                    nm, rstd = _ln_stats(nc, pA2, sqpool, rowp2, ones_col,
                                         h, cs, "nm", "rstd", eps_row, rowp4,
                                         pstag="score")

                    # rstd broadcast [128, S] (SBUF, reused 12x+)
                    bc = pA3.tile([128, S], F32, tag="obc")
                    nc.tensor.matmul(bc[:], _r(ones_row[:]), _r(rstd[:]))
                    b1s = brp.tile([128, S], F32, tag="b1s")
                    nc.vector.tensor_copy(b1s[:], bc[:])

                    # rstd as a column [128, BL] for per-partition V scaling
                    rc_ps = pA2.tile([128, BL], F32, tag="score")
                    for g in range(BL):
                        nc.tensor.matmul(rc_ps[:, g:g + 1],
                                         rstd[0:1, g * 128:
                                              (g + 1) * 128].bitcast(F32),
                                         ident1[:], is_transpose=True)
                    rcol = brp.tile([128, BL], F32, tag="rcol")
                    nc.vector.tensor_copy(rcol[:], rc_ps[:])

                    # V in natural layout, rstd-scaled, into [V|1] aug slots
                    for g in range(BL):
                        tok = slice(c * S + g * 128, c * S + (g + 1) * 128)
                        for half in range(2):
                            n0 = half * 384
                            ps = pA2.tile([128, 384], F32, tag="qkv")
                            for kt in range(KT):
                                nc.tensor.matmul(
                                    ps[:], _r(h[:, kt, tok]),
                                    _r(wv[:, kt, n0:n0 + 384]),
                                    start=(kt == 0), stop=False)
                            nc.tensor.matmul(
                                ps[:], _r(nm[0:1, g * 128:(g + 1) * 128]),
                                _r(cs_sb[0:1, 2 * D + n0:2 * D + n0 + 384]),
                                start=False, stop=True)
                            h0 = half * 6
                            nc.vector.tensor_scalar(
                                vaug[:, g, h0:h0 + 6, 0:DH],
                                ps[:].rearrange("p (h d) -> p h d", d=DH),
                                rcol[:, g:g + 1], None, OP.mult)

                    # per n-tile: Q, K, then attention for heads 2n, 2n+1
                    oT = oTp.tile([128, KT, S], F32R, tag="oT")
                    for n in range(KT):
                        wqs = wqp.tile([128, KT, 128], F32R, tag="wqs")
                        nc.sync.dma_start(wqs[:], d["wq"][lyr, n])
                        wks = wqp.tile([128, KT, 128], F32R, tag="wks")
                        nc.sync.dma_start(wks[:], d["wk"][lyr, n])
                        qs = qkp.tile([128, S], F32R, tag="qs")
                        ks = qkp.tile([128, S], F32R, tag="ks")
                        for dst, w, csoff in ((qs, wqs, 0), (ks, wks, D)):
                            ps = pA2.tile([128, S], F32, tag="qkv")
                            for kt in range(KT):
                                nc.tensor.matmul(
                                    ps[:], _r(w[:, kt, :]), _r(h[:, kt, cs]),
                                    start=(kt == 0), stop=False)
                            nc.tensor.matmul(
                                ps[:], _r(cs_sb[0:1, csoff + n * 128:
                                                 csoff + (n + 1) * 128]),
                                _r(nm[:]), start=False, stop=True)
                            nc.vector.tensor_tensor(dst[:], ps[:], b1s[:],
                                                    OP.mult)

                        for sub in range(2):
                            hd = 2 * n + sub
                            pb = 64 * sub
                            o_ps = pA3.tile([DH + 1, S], F32, tag="obc")
                            at_tiles = []
                            for j in range(BL):
                                q0 = j * 128
                                sp = pA2.tile([128, S], F32, tag="score")
                                nc.tensor.matmul(
                                    sp[:, q0:],
                                    ks[pb:pb + DH,
                                       j * 128:(j + 1) * 128].bitcast(F32R),
                                    qs[pb:pb + DH, q0:].bitcast(F32R),
                                    start=True, stop=False)
                                nc.tensor.matmul(
                                    sp[:, q0:q0 + 128], i128b[:],
                                    (mask00 if j == 0 else maskdg)[:],
                                    start=False, stop=True)
                                at = atp.tile([128, S], F32R, tag="at")
                                nc.scalar.activation(
                                    at[:, q0:], sp[:, q0:], AF.Exp,
                                    bias=padsb[:, c * BL + j:c * BL + j + 1])
                                at_tiles.append(at)
                            for j in range(BL):
                                q0 = j * 128
                                nc.tensor.matmul(
                                    o_ps[:, q0:], _r(vaug[:, j, hd, :]),
                                    _r(at_tiles[j][:, q0:]),
                                    start=(j == 0), stop=(j == BL - 1))
                            inv = rowp2.tile([1, S], F32R, tag="inv")
                            with nc.allow_low_precision(
                                    reason="f32r softmax denom (19-bit ok)"):
                                nc.vector.reciprocal(inv[:],
                                                     o_ps[DH:DH + 1, :])
                            bco = pA3.tile([DH, S], F32, tag="obc")
                            nc.tensor.matmul(bco[:], _r(ones_row[0:1, 0:DH]),
                                             _r(inv[:]))
                            bcos = brp1.tile([DH, S], F32, tag="bcos")
                            nc.scalar.activation(bcos[:], bco[:], AF.Copy)
                            nc.vector.tensor_tensor(
                                oT[pb:pb + DH, n, :], o_ps[0:DH, :], bcos[:],
                                OP.mult)

                    # Wo + residual
                    for n in range(KT):
                        ps = pA2.tile([128, S], F32, tag="qkv")
                        for kt in range(KT):
                            nc.tensor.matmul(
                                ps[:], _r(wo[:, kt, n * 128:(n + 1) * 128]),
                                _r(oT[:, kt, :]),
                                start=(kt == 0), stop=(kt == KT - 1))
                        if has_bo:
                            tmo = tmpA.tile([128, S], F32, tag="ft")
                            nc.vector.tensor_scalar(
                                tmo[:], ps[:], bo_c[:, lyr, n:n + 1],
                                None, OP.add)
                            nc.vector.tensor_tensor(h[:, n, cs], h[:, n, cs],
                                                    tmo[:], OP.add)
                        else:
                            nc.vector.tensor_tensor(h[:, n, cs], h[:, n, cs],
                                                    ps[:], OP.add)

                    # LN2 stats for this chunk (reuses phase-A psum scope)
                    nm2, rstd2 = _ln_stats(nc, pA2, sqpool, rownm2, ones_col,
                                           h, cs, "nm2", "rstd2", eps_row,
                                           rowp4, pstag="score")
                    bc2 = pA3.tile([128, S], F32, tag="obc")
                    nc.tensor.matmul(bc2[:], _r(ones_row[:]), _r(rstd2[:]))
                    br2 = b2spool.tile([128, S], F32, tag="br2")
                    nc.vector.tensor_copy(br2[:], bc2[:])
                    nm2s.append(nm2)
                    br2s.append(br2)

            # ---------- phase B: FFN ----------
            with tc.tile_pool(name=f"pD_{lyr}", bufs=1, space="PSUM") as pB, \
                 tc.tile_pool(name=f"pE_{lyr}", bufs=2, space="PSUM") as pB2, \
                 tc.tile_pool(name=f"w1_{lyr}", bufs=3) as w1p, \
                 tc.tile_pool(name=f"w2_{lyr}", bufs=3) as w2p, \
                 tc.tile_pool(name=f"rl_{lyr}", bufs=3) as rlp, \
                 tc.tile_pool(name=f"c1_{lyr}", bufs=1) as c1p, \
                 tc.tile_pool(name=f"tf_{lyr}", bufs=3) as tfp:
                cs1_sb = c1p.tile([1, FF], F32R, tag="cs1")
                nc.sync.dma_start(cs1_sb[:], d["cs1"][lyr])
                for c in range(BL):
                    cs = slice(c * S, (c + 1) * S)
                    out_ps = pB.tile([128, KT, S], F32, tag="fout")
                    for f in range(FT):
                        w1 = w1p.tile([128, KT, 128], F32R, tag="w1")
                        nc.sync.dma_start(w1[:], d["w1t"][lyr, f])
                        w2 = w2p.tile([128, D], F32R, tag="w2")
                        nc.sync.dma_start(w2[:], d["w2t"][lyr, f])
                        ps1 = pB2.tile([128, S], F32, tag="w1ps")
                        for kt in range(KT):
                            nc.tensor.matmul(ps1[:], _r(w1[:, kt, :]),
                                             _r(h[:, kt, cs]),
                                             start=(kt == 0), stop=False)
                        nc.tensor.matmul(
                            ps1[:], _r(cs1_sb[0:1, f * 128:(f + 1) * 128]),
                            _r(nm2s[c][:]), start=False, stop=True)
                        rl = rlp.tile([128, S], F32R, tag="rl")
                        nc.scalar.activation(rl[:], ps1[:], AF.Relu)
                        for n in range(KT):
                            nc.tensor.matmul(
                                out_ps[:, n, :],
                                _r(w2[:, n * 128:(n + 1) * 128]), _r(rl[:]),
                                start=(f == 0), stop=(f == FT - 1))
                    for n in range(KT):
                        tmp = tfp.tile([128, S], F32, tag="ft")
                        nc.vector.tensor_tensor(tmp[:], out_ps[:, n, :],
                                                br2s[c][:], OP.mult)
                        if has_b2:
                            nc.vector.tensor_scalar(
                                tmp[:], tmp[:], b2_c[:, lyr, n:n + 1], None,
                                OP.add)
                        nc.vector.tensor_tensor(h[:, n, cs], h[:, n, cs],
                                                tmp[:], OP.add)

        # ================= output transpose =================
        with tc.tile_pool(name="outsb", bufs=2) as osb, \
             tc.tile_pool(name="outps", bufs=6, space="PSUM") as ops:
            for g in range(G16):
                ob = osb.tile([128, D], F32, tag="ob")
                for kt in range(KT):
                    p = ops.tile([128, 128], F32, tag="otp")
                    nc.tensor.matmul(p[:],
                                     h[:, kt,
                                       g * 128:(g + 1) * 128].bitcast(F32),
                                     i128f[:], is_transpose=True)
                    nc.vector.tensor_copy(ob[:, kt * 128:(kt + 1) * 128],
                                          p[:])
                nc.sync.dma_start(d_out[g * 128:(g + 1) * 128, :], ob[:])


# ======================= host side =======================

def _pos_enc():
    pos = np.arange(S, dtype=np.float32)[:, None]
    i = np.arange(0, D, 2, dtype=np.float32)[None, :]
    ang = pos / np.power(10000.0, i / D)
    pe = np.zeros((S, D), dtype=np.float32)
    pe[:, 0::2] = np.sin(ang)
    pe[:, 1::2] = np.cos(ang)
    return pe


def _idx16(v):
    """dma_gather index layout: idx i at [i % 16, i // 16].
    CoreSim's ucode model reads partitions 0..15; the deployed HW ucode reads
    partitions 16..31 — write both ranges so either consumer sees the same
    indices."""
    arr = np.zeros((128, T // 16), np.int16)
    w = v.reshape(T // 16, 16).T.astype(np.int16)
    arr[:16, :] = w
    arr[16:32, :] = w
    return arr


_NC_CACHE = {}


def _get_nc(nl, has_bo, has_b2, has_embgb):
    key = (nl, has_bo, has_b2, has_embgb)
    if key not in _NC_CACHE:
        _NC_CACHE[key] = build_nc(nl, has_bo, has_b2, has_embgb)
    return _NC_CACHE[key]


def prepare(inputs, n_layers=None):
    """Host-side preprocessing -> (flags, shared input map, per-core maps)."""
    x = np.asarray(inputs["x"])
    time_t = np.asarray(inputs["time"])
    len_traj = int(np.asarray(inputs["len_traj"]))
    tok_emb = np.asarray(inputs["tok_emb"], np.float32)
    time_emb = np.asarray(inputs["time_emb"], np.float32)
    emb_g = np.asarray(inputs["emb_g"], np.float32)
    emb_b = np.asarray(inputs["emb_b"], np.float32)
    Wq = np.asarray(inputs["Wq"], np.float32)
    bq = np.asarray(inputs["bq"], np.float32)
    Wk = np.asarray(inputs["Wk"], np.float32)
    bk = np.asarray(inputs["bk"], np.float32)
    Wv = np.asarray(inputs["Wv"], np.float32)
    bv = np.asarray(inputs["bv"], np.float32)
    Wo = np.asarray(inputs["Wo"], np.float32)
    bo = np.asarray(inputs["bo"], np.float32)
    ln1_g = np.asarray(inputs["ln1_g"], np.float32)
    ln1_b = np.asarray(inputs["ln1_b"], np.float32)
    W1 = np.asarray(inputs["W1"], np.float32)
    b1 = np.asarray(inputs["b1"], np.float32)
    W2 = np.asarray(inputs["W2"], np.float32)
    b2 = np.asarray(inputs["b2"], np.float32)
    ln2_g = np.asarray(inputs["ln2_g"], np.float32)
    ln2_b = np.asarray(inputs["ln2_b"], np.float32)

    nl = Wq.shape[0] if n_layers is None else n_layers
    Wq, Wk, Wv, Wo = Wq[:nl], Wk[:nl], Wv[:nl], Wo[:nl]
    bq, bk, bv, bo = bq[:nl], bk[:nl], bv[:nl], bo[:nl]
    W1, b1, W2, b2 = W1[:nl], b1[:nl], W2[:nl], b2[:nl]
    ln1_g, ln1_b, ln2_g, ln2_b = ln1_g[:nl], ln1_b[:nl], ln2_g[:nl], ln2_b[:nl]

    scale = np.float32(1.0 / np.sqrt(DH))

    # fold LN gains/betas into adjacent weights (exact)
    Wq_f = ln1_g[:, :, None] * Wq * scale
    Wk_f = ln1_g[:, :, None] * Wk
    Wv_f = ln1_g[:, :, None] * Wv
    bq_f = np.einsum("ld,ldn->ln", ln1_b, Wq) * scale + bq * scale
    bk_f = np.einsum("ld,ldn->ln", ln1_b, Wk) + bk
    bv_f = np.einsum("ld,ldn->ln", ln1_b, Wv) + bv
    W1_f = ln2_g[:, :, None] * W1
    b1_f = np.einsum("ld,ldn->ln", ln2_b, W1) + b1

    def _mx(a):
        return np.abs(a).max() if a.size else 0.0

    unsupported = []
    if _mx(bq_f) > 0 or _mx(bk_f) > 0:
        unsupported.append("bq/bk")
    if _mx(bv_f) > 0:
        unsupported.append("bv")
    if _mx(b1_f) > 0:
        unsupported.append("b1")
    if unsupported:
        raise NotImplementedError(f"nonzero folded biases: {unsupported}")

    has_bo = bool(_mx(bo) > 0)
    has_b2 = bool(_mx(b2) > 0)
    has_embgb = bool(np.abs(emb_g - 1).max() > 0 or np.abs(emb_b).max() > 0)

    csq = Wq_f.sum(axis=1)
    csk = Wk_f.sum(axis=1)
    csv = Wv_f.sum(axis=1)
    cs1 = W1_f.sum(axis=1)

    def qlay(w):  # [L, D, N] -> [L, 128, KT, N]
        return np.ascontiguousarray(
            w.reshape(nl, KT, 128, w.shape[2]).transpose(0, 2, 1, 3))

    def nlay(w):  # [L, D, N] -> [L, N/128, 128(d sub), D/128, 128(n sub)]
        nt = w.shape[2] // 128
        return np.ascontiguousarray(
            w.reshape(nl, KT, 128, nt, 128).transpose(0, 3, 2, 1, 4))

    shared = {
        "tok_emb": tok_emb, "tim_emb": time_emb,
        "wq": nlay(Wq_f), "wk": nlay(Wk_f), "wv": qlay(Wv_f), "wo": qlay(Wo),
        "w1t": nlay(W1_f),
        "w2t": np.ascontiguousarray(W2.reshape(nl, FT, 128, D)),
        "csqkv": np.ascontiguousarray(
            np.concatenate([csq, csk, csv], axis=1).reshape(nl, 1, 3 * D)),
        "cs1": np.ascontiguousarray(cs1.reshape(nl, 1, FF)),
    }

    pe = _pos_enc()
    shared["posT"] = np.ascontiguousarray(
        pe.T.reshape(KT, 128, S).transpose(1, 0, 2))

    ii = np.arange(128)
    tril = (ii[None, :] >= ii[:, None])  # [k, q]: q >= k
    shared["maskdg"] = np.where(tril, 0.0, NEGM).astype(ml_dtypes.bfloat16)
    shared["mask00"] = np.where(tril | (ii[:, None] < len_traj), 0.0,
                                NEGM).astype(ml_dtypes.bfloat16)
    eye = np.eye(128, dtype=np.float32)
    shared["i128b"] = eye.astype(ml_dtypes.bfloat16)
    shared["i128f"] = eye

    if has_bo:
        shared["bo_c"] = np.ascontiguousarray(
            bo.reshape(nl, KT, 128).transpose(2, 0, 1))
    if has_b2:
        shared["b2_c"] = np.ascontiguousarray(
            b2.reshape(nl, KT, 128).transpose(2, 0, 1))
    if has_embgb:
        eg = np.zeros((128, KT, 2), np.float32)
        eg[:, :, 0] = emb_g.reshape(KT, 128).T
        eg[:, :, 1] = emb_b.reshape(KT, 128).T
        shared["embg"] = eg

    in_maps = []
    for core in range(N_CORES):
        bs = slice(core * BL, (core + 1) * BL)
        xl = np.asarray(x[bs]).reshape(-1)
        tl = np.asarray(time_t[bs]).reshape(-1)
        pad = np.where(xl > 0, 0.0, NEGM).astype(np.float32)
        m = dict(shared)
        m["tok_idx"] = _idx16(xl)
        m["tim_idx"] = _idx16(tl)
        m["padadd"] = np.ascontiguousarray(pad.reshape(G16, 128).T)
        in_maps.append(m)

    return (nl, has_bo, has_b2, has_embgb), in_maps


def run(inputs, n_layers=None, **run_kwargs):
    """Run on hardware; returns (output [32, 512, 768], BassKernelResults)."""
    key, in_maps = prepare(inputs, n_layers)
    nc = _get_nc(*key)
    res = bass_utils.run_bass_kernel_spmd(nc, in_maps,
                                          core_ids=list(range(N_CORES)),
                                          **run_kwargs)
    outs = [res.results[i]["out"] for i in range(N_CORES)]
    full = np.concatenate([np.asarray(o).reshape(BL, S, D) for o in outs],
                          axis=0)
    return full.astype(np.float32), res


def kernel(**inputs):
    out, _ = run(inputs)
    return out

